# revision 56
# baseline (speedup 1.0000x reference)
"""Multi-head attention block (QKV proj + softmax attention + out proj) on 8
Trainium2 NeuronCores, data-parallel over the batch dimension (one batch
element per core).

Self-contained: hardcodes shapes for x [8, 1024, 768], qkv_w [768, 2304],
proj_w [768, 768], proj_b [768]; returns [8, 1024, 768] float32.

Layout/schedule notes:
- All matmul operands are bf16 (host pre-converts); psum stays f32.
- Host pre-arranges every input into its SBUF image ([partition, ...]) so
  each load is one large fully-coalesced DMA.
- Per-head-pair software pipeline: window hp emits SC(hp+1) + AV(hp) + a
  slice of QK(hp+2) on the PE queue so the scalar-engine exp stream (the
  2nd-largest serial cost) hides completely behind matmuls.
- Softmax denominators ride as a 65th row of the V stationary tiles
  (ones-augmented), then reciprocal on DVE + gpsimd partition_broadcast;
  the normalize multiply reads the AV psum directly (no evacuation copy).
"""

import numpy as np
import ml_dtypes

import concourse.bass as bass
import concourse.mybir as mybir
import concourse.tile as tile
from concourse import bacc

N_CORES = 8
N = 1024          # tokens per batch element
C = 768           # model dim
H = 12            # heads
HD = 64           # head dim
CT = C // 128     # 6 contraction tiles
TT = N // 128     # 8 token tiles
HP = H // 2       # 6 head pairs
SCALE = HD ** -0.5

F32 = mybir.dt.float32
BF16 = mybir.dt.bfloat16


def _build():
    nc = bacc.Bacc("TRN2", target_bir_lowering=False, debug=False,
                   num_devices=N_CORES)
    xt = nc.dram_tensor("xt_sb", [128, CT, N], BF16, kind="ExternalInput").ap()
    wqk = nc.dram_tensor("wqk_sb", [128, CT, 2 * C], BF16, kind="ExternalInput").ap()
    wv = nc.dram_tensor("wv_sb", [128, CT, C], BF16, kind="ExternalInput").ap()
    pw = nc.dram_tensor("pw_sb", [128, CT, C], BF16, kind="ExternalInput").ap()
    pb = nc.dram_tensor("pb", [1, C], F32, kind="ExternalInput").ap()
    out = nc.dram_tensor("out", [N, C], BF16, kind="ExternalOutput").ap()

    with tile.TileContext(nc) as tc:
        _emit(nc, tc, xt, wqk, wv, pw, pb, out)
    nc.compile()
    return nc


def _emit(nc, tc, xt, wqk, wv, pw, pb, out):
    from contextlib import ExitStack
    ctx = ExitStack()
    with ctx:
        sb = ctx.enter_context(tc.tile_pool(name="sb", bufs=1))
        epool = ctx.enter_context(tc.tile_pool(name="ep", bufs=12))
        upool = ctx.enter_context(tc.tile_pool(name="up", bufs=2))
        npool = ctx.enter_context(tc.tile_pool(name="norm", bufs=2))
        opool = ctx.enter_context(tc.tile_pool(name="osb", bufs=2))
        ps = ctx.enter_context(tc.tile_pool(name="scps", bufs=2, space="PSUM"))
        av_ps = ctx.enter_context(tc.tile_pool(name="avps", bufs=1, space="PSUM"))
        dpool = ctx.enter_context(tc.tile_pool(name="drs", bufs=2, space="DRAM"))

        XT = sb.tile([128, CT, N], BF16, tag="XT")
        WQK = sb.tile([128, CT, 2 * C], BF16, tag="WQK")
        WV = sb.tile([128, CT, C], BF16, tag="WV")
        PWt = sb.tile([128, CT, C], BF16, tag="PW")
        QT = sb.tile([128, HP, N], BF16, tag="QT")
        KT = sb.tile([128, HP, N], BF16, tag="KT")
        VA = sb.tile([128, TT, H, HD + 1], BF16, tag="VA")
        OT = sb.tile([128, CT, N], BF16, tag="OT")
        PBB = sb.tile([128, C], F32, tag="PBB")

        # ---- input loads: two HW DMA queues (sync + scalar) in parallel.
        # The critical-path tensors (WQK slice 0, XT, WV) are split across
        # both queues up front; everything else trails on the scalar queue.
        nc.sync.dma_start(WQK[:, :, 0:256], wqk[:, :, 0:256])
        nc.sync.dma_start(XT[:], xt[:])
        nc.sync.dma_start(WV[:], wv[:])
        for hp in range(1, HP):
            nc.scalar.dma_start(WQK[:, :, hp * 256:(hp + 1) * 256],
                                wqk[:, :, hp * 256:(hp + 1) * 256])
        nc.scalar.dma_start(PWt[:], pw[:])
        pbsrc = pb[:, :]
        pb_b = bass.AP(tensor=pbsrc.tensor, offset=pbsrc.offset,
                       ap=[[0, 128]] + [list(a) for a in pbsrc.ap[1:]])
        nc.gpsimd.dma_start(PBB[:], pb_b)
        nc.vector.memset(
            VA[:, :, :, HD:HD + 1].rearrange("p t h o -> p (t h o)"), 1.0)

        e_tiles = {}

        def emit_qk_unit(hp, u):
            qk, qc = divmod(u, 2)
            dest = QT if qk == 0 else KT
            f0 = hp * 256 + qk * 128
            p = ps.tile([128, 512], F32, tag="sc", name="qkps")
            for ct in range(CT):
                nc.tensor.matmul(
                    p[:], lhsT=WQK[:, ct, f0:f0 + 128],
                    rhs=XT[:, ct, qc * 512:(qc + 1) * 512],
                    start=(ct == 0), stop=(ct == CT - 1))
            nc.vector.tensor_copy(dest[:, hp, qc * 512:(qc + 1) * 512], p[:])

        def emit_v_unit(tt, half):
            p = ps.tile([128, 384], F32, tag="sc", name="vps")
            for ct in range(CT):
                nc.tensor.matmul(
                    p[:], lhsT=XT[:, ct, tt * 128:(tt + 1) * 128],
                    rhs=WV[:, ct, half * 384:(half + 1) * 384],
                    start=(ct == 0), stop=(ct == CT - 1))
            nc.vector.tensor_copy(
                VA[:, tt, half * 6:(half + 1) * 6, 0:HD],
                p[:].rearrange("p (h d) -> p h d", d=HD))

        def emit_sc_pair(w, kt):
            # scores for both heads of pair w, packed [A-qc | B-qc] per tile:
            # A runs on PE rows 0-63, B on rows 64-127 — adjacent row-tiled
            # matmuls dual-stream (~1.7x measured vs serial)
            es = []
            for qc in range(2):
                pS = ps.tile([128, N], F32, tag="sc", name="scps")
                nc.tensor.matmul(
                    pS[:, 0:512],
                    lhsT=KT[0:64, w, kt * 128:(kt + 1) * 128],
                    rhs=QT[0:64, w, qc * 512:(qc + 1) * 512],
                    start=True, stop=True)
                nc.tensor.matmul(
                    pS[:, 512:1024],
                    lhsT=KT[64:128, w, kt * 128:(kt + 1) * 128],
                    rhs=QT[64:128, w, qc * 512:(qc + 1) * 512],
                    start=True, stop=True)
                e = epool.tile([128, N], BF16, tag="e", name="e")
                nc.scalar.activation(e[:], pS[:],
                                     mybir.ActivationFunctionType.Exp,
                                     scale=SCALE)
                es.append(e)
            e_tiles[(w, kt)] = es

        def emit_av_pair(w, kt, avA, avB):
            e0, e1 = e_tiles.pop((w, kt))
            for qc, e in ((0, e0), (1, e1)):
                nc.tensor.matmul(
                    avA[:, qc * 512:(qc + 1) * 512],
                    lhsT=VA[:, kt, 2 * w, :],
                    rhs=e[:, 0:512],
                    start=(kt == 0), stop=(kt == TT - 1))
                nc.tensor.matmul(
                    avB[:, qc * 512:(qc + 1) * 512],
                    lhsT=VA[:, kt, 2 * w + 1, :],
                    rhs=e[:, 512:1024],
                    start=(kt == 0), stop=(kt == TT - 1))

        def emit_norm(h, av):
            hp, half = divmod(h, 2)
            if h == H - 1:
                # last head is latency-critical (proj finals wait on it):
                # skip the DMA transpose round-trip; reciprocal straight off
                # the psum row, broadcast across partitions on gpsimd
                d0 = npool.tile([1, N], F32, tag="d0", name="d0")
                nc.vector.tensor_copy(d0[:], av[HD:HD + 1, :])
                U = upool.tile([HD + 1, N], F32, tag="U", name="U")
                nc.vector.tensor_copy(U[:], av[:])
                Db = npool.tile([64, N], F32, tag="Db", name="Db")
                nc.gpsimd.partition_broadcast(Db[:], d0[0:1, :])
                bc = npool.tile([64, N], F32, tag="bc", name="bc")
                # fast variant (~2^-12 rel err, plenty here): one DVE pass
                # less on the latency-critical last-head chain
                nc.vector.reciprocal_approx_fast(out=bc[:], in_=Db[:])
                for qc in range(2):
                    nc.vector.tensor_mul(
                        OT[half * 64:(half + 1) * 64, hp,
                           qc * 512:(qc + 1) * 512],
                        U[0:HD, qc * 512:(qc + 1) * 512],
                        bc[:, qc * 512:(qc + 1) * 512])
                return
            # evacuate psum right away so the next window's AV can allocate;
            # the reciprocal chain then runs from SBUF asynchronously
            U = upool.tile([HD + 1, N], F32, tag="U", name="U")
            nc.vector.tensor_copy(U[:], av[:])
            dscr = dpool.tile([N], F32, tag="dscr", name="dscr")
            nc.sync.dma_start(dscr[:], U[HD:HD + 1, :])
            Dt = npool.tile([64, 16], F32, tag="Dt", name="Dt")
            nc.sync.dma_start(Dt[:], dscr[:].rearrange("(p j) -> p j", j=16))
            Rt = npool.tile([64, 16], F32, tag="Rt", name="Rt")
            scr = npool.tile([64, 16], F32, tag="scr", name="scr")
            nc.vector.reciprocal_approx_accurate(Rt[:], Dt[:], scr[:])
            rscr = dpool.tile([N], F32, tag="rscr", name="rscr")
            nc.sync.dma_start(rscr[:].rearrange("(p j) -> p j", j=16), Rt[:])
            bc = npool.tile([64, N], F32, tag="bc", name="bc")
            rs = rscr[:]
            bcast = bass.AP(tensor=rs.tensor, offset=rs.offset,
                            ap=[[0, 64]] + [list(a) for a in rs.ap])
            nc.sync.dma_start(bc[:], bcast)
            for qc in range(2):
                nc.vector.tensor_mul(
                    OT[half * 64:(half + 1) * 64, hp, qc * 512:(qc + 1) * 512],
                    U[0:HD, qc * 512:(qc + 1) * 512],
                    bc[:, qc * 512:(qc + 1) * 512])

        osb_tiles = {}

        def _pj_views(p, osb):
            # matmul outs go to cols [0:384] and [512:896] (bank-aligned);
            # one strided DVE add folds both into the packed osb layout
            pv = p[:, 0:1024].rearrange("p (b k) -> p b k", b=2)[:, :, 0:384]
            ov = osb[:, 0:768].rearrange("p (b k) -> p b k", b=2)
            return pv, ov

        def emit_proj_partial(tt):
            # accumulate proj over ct 0..4 (heads 0..9, normalized well before
            # the last window) into SBUF, leaving only ct 5 for the tail
            osb = osb_tiles[tt] = opool.tile([128, C], BF16, tag="osb",
                                             name="osb", bufs=TT)
            p = ps.tile([128, N], F32, tag="sc", name="pjps")
            for nch in range(2):
                for ct in range(CT - 1):
                    nc.tensor.matmul(
                        p[:, nch * 512:nch * 512 + 384],
                        lhsT=OT[:, ct, tt * 128:(tt + 1) * 128],
                        rhs=PWt[:, ct, nch * 384:(nch + 1) * 384],
                        start=(ct == 0), stop=(ct == CT - 2))
            pv, ov = _pj_views(p, osb)
            bv = PBB[:, 0:768].rearrange("p (b k) -> p b k", b=2)
            nc.vector.tensor_add(ov, pv, bv)

        def emit_proj_final(tt, q1):
            osb = osb_tiles[tt]
            # alternate between the two free psum rings so final matmuls
            # pipeline deeper instead of stalling behind each DVE add
            pool, tag = (ps, "sc") if tt % 2 == 0 else (av_ps, "avA")
            p = pool.tile([128, N], F32, tag=tag, name="pjps")
            for nch in range(2):
                nc.tensor.matmul(
                    p[:, nch * 512:nch * 512 + 384],
                    lhsT=OT[:, CT - 1, tt * 128:(tt + 1) * 128],
                    rhs=PWt[:, CT - 1, nch * 384:(nch + 1) * 384],
                    start=True, stop=True)
            pv, ov = _pj_views(p, osb)
            nc.vector.tensor_add(ov, ov, pv)
            eng = nc.sync if q1 else nc.scalar
            eng.dma_start(out[tt * 128:(tt + 1) * 128, :], osb[:])

        # filler units per (window, kt-group): V projections + next QK
        # projections slotted into PE slack so no phase runs standalone.
        # Deadlines: V half-0 unit tt before AV(0,tt); V half-1 before w3;
        # QK(w+1) during window w; proj partials after norm(9) = window 4.
        # group TT (=8) is a boundary slot consumed after the window's AV
        # drain: it papers over the exp(kt7)->SC(next,kt0) ring bubble
        fillers = {}
        for kt in range(TT):
            fillers[(0, kt)] = [lambda kt=kt: emit_v_unit(kt, 0)]
            if kt % 2 == 1 and kt < 7:
                fillers[(0, kt)].append(
                    lambda u=kt // 2: emit_qk_unit(1, u))
        fillers[(0, TT)] = [lambda: emit_qk_unit(1, 3)]
        for w in (1, 2):
            for j in range(4):
                fillers.setdefault((w, 2 * j + 1), []).append(
                    lambda tt=4 * (w - 1) + j: emit_v_unit(tt, 1))
            for j in range(3):
                fillers.setdefault((w, 2 * j), []).append(
                    lambda hp=w + 1, u=j: emit_qk_unit(hp, u))
            fillers[(w, TT)] = [lambda hp=w + 1: emit_qk_unit(hp, 3)]
        for w in (3, 4):
            for j in range(3):
                fillers.setdefault((w, 2 * j), []).append(
                    lambda hp=w + 1, u=j: emit_qk_unit(hp, u))
            fillers[(w, TT)] = [lambda hp=w + 1: emit_qk_unit(hp, 3)]
        for j in range(4):
            fillers.setdefault((5, 4 + j), []).append(
                lambda tt=j: emit_proj_partial(tt))

        # ---- schedule: PE warm-up, lead-in QK(0), one head-pair/window ----
        # dummy matmuls while the input DMAs land: the PE clock needs ~3us
        # of continuous work to ramp to full speed, so burn the DMA wait on
        # throwaway accumulates instead of ramping inside the first window
        warm = sb.tile([128, 512], BF16, tag="warm")
        nc.vector.memset(warm[:], 0.25)
        wp = ps.tile([128, 512], F32, tag="sc", name="warmps")
        for i in range(26):
            # 64-row contraction: keeps the clock ramping at half the MAC
            # power (sustained full-array warm work trips the util throttle)
            nc.tensor.matmul(wp[:], lhsT=warm[0:64, 0:128], rhs=warm[0:64, :],
                             start=(i == 0), stop=(i == 25))
        for u in range(4):
            emit_qk_unit(0, u)
        for w in range(HP):
            avA = av_ps.tile([HD + 1, N], F32, tag="avA", name="avA")
            avB = av_ps.tile([HD + 1, N], F32, tag="avB", name="avB")
            pend = []
            for kt in range(TT):
                emit_sc_pair(w, kt)
                pend.append(kt)
                if len(pend) > 2:
                    emit_av_pair(w, pend.pop(0), avA, avB)
                for f in fillers.get((w, kt), ()):
                    f()
            # drain: slot the boundary filler between the last two AV pairs
            # so it covers the exp(kt7) wait instead of trailing it
            emit_av_pair(w, pend[0], avA, avB)
            for f in fillers.get((w, TT), ()):
                f()
            emit_av_pair(w, pend[1], avA, avB)
            emit_norm(2 * w, avA)
            emit_norm(2 * w + 1, avB)
        for tt in range(4, TT):
            emit_proj_partial(tt)
        for tt in range(TT):
            emit_proj_final(tt, tt % 2 == 0)


_CACHE = {}


def _get_runner():
    """Build + compile once; return a callable(in_maps) -> list of out dicts.

    Keeps a persistent jitted shard_map executable so repeat calls skip
    retracing/recompiling (mirrors bass2jax.run_bass_via_pjrt).
    """
    if "runner" in _CACHE:
        return _CACHE["runner"]

    import jax
    from jax.experimental.shard_map import shard_map
    from jax.sharding import Mesh, PartitionSpec
    from concourse import bass2jax

    nc = _build()
    bass2jax.install_neuronx_cc_hook()

    partition_name = (nc.partition_id_tensor.name if nc.partition_id_tensor
                      else None)
    in_names, out_names, out_avals, zero_outs = [], [], [], []
    for alloc in nc.m.functions[0].allocations:
        if not isinstance(alloc, mybir.MemoryLocationSet):
            continue
        name = alloc.memorylocations[0].name
        if alloc.kind == "ExternalInput":
            if name != partition_name:
                in_names.append(name)
        elif alloc.kind == "ExternalOutput":
            out_names.append(name)
            shape = tuple(alloc.tensor_shape)
            dtype = mybir.dt.np(alloc.dtype)
            out_avals.append(jax.core.ShapedArray(shape, dtype))
            zero_outs.append(np.zeros(shape, dtype))
    n_params = len(in_names)
    n_outs = len(out_avals)
    all_in_names = list(in_names) + list(out_names)
    if partition_name is not None:
        all_in_names.append(partition_name)
    donate = tuple(range(n_params, n_params + n_outs))

    def _body(*args):
        operands = list(args)
        if partition_name is not None:
            operands.append(bass2jax.partition_id_tensor())
        outs = bass2jax._bass_exec_p.bind(
            *operands,
            out_avals=tuple(out_avals),
            in_names=tuple(all_in_names),
            out_names=tuple(out_names),
            lowering_input_output_aliases=(),
            sim_require_finite=True,
            sim_require_nnan=True,
            nc=nc,
        )
        return tuple(outs)

    devices = jax.devices()[:N_CORES]
    mesh = Mesh(np.asarray(devices), ("core",))
    in_specs = (PartitionSpec("core"),) * (n_params + n_outs)
    out_specs = (PartitionSpec("core"),) * n_outs
    sharded = jax.jit(
        shard_map(_body, mesh=mesh, in_specs=in_specs, out_specs=out_specs,
                  check_rep=False),
        donate_argnums=donate, keep_unused=True)

    def runner(in_maps):
        concat_in = [
            np.concatenate([np.asarray(m[name]) for m in in_maps], axis=0)
            for name in in_names
        ]
        concat_zeros = [
            np.zeros((N_CORES * z.shape[0], *z.shape[1:]), z.dtype)
            for z in zero_outs
        ]
        out_arrs = sharded(*concat_in, *concat_zeros)
        return [
            {name: np.asarray(out_arrs[i]).reshape(N_CORES, *out_avals[i].shape)[c]
             for i, name in enumerate(out_names)}
            for c in range(N_CORES)
        ]

    _CACHE["runner"] = runner
    _CACHE["nc"] = nc
    return runner


def make_in_maps(x, qkv_w, proj_w, proj_b):
    bf = ml_dtypes.bfloat16
    qkv = np.asarray(qkv_w, np.float32)
    q = qkv[:, 0:C].reshape(CT, 128, HP, 128)
    k = qkv[:, C:2 * C].reshape(CT, 128, HP, 128)
    wqk_sb = np.empty((128, CT, 2 * C), np.float32)
    for hp in range(HP):
        wqk_sb[:, :, hp * 256:hp * 256 + 128] = q[:, :, hp, :].transpose(1, 0, 2)
        wqk_sb[:, :, hp * 256 + 128:hp * 256 + 256] = \
            k[:, :, hp, :].transpose(1, 0, 2)
    wqk_sb = np.ascontiguousarray(wqk_sb).astype(bf)
    wv_sb = np.ascontiguousarray(
        qkv[:, 2 * C:3 * C].reshape(CT, 128, C).transpose(1, 0, 2)).astype(bf)
    pw_sb = np.ascontiguousarray(
        np.asarray(proj_w, np.float32).reshape(CT, 128, C)
        .transpose(1, 0, 2)).astype(bf)
    pbarr = np.asarray(proj_b, np.float32).reshape(1, C)
    maps = []
    for b in range(N_CORES):
        xt_sb = np.ascontiguousarray(
            np.asarray(x[b], np.float32).T.reshape(CT, 128, N)
            .transpose(1, 0, 2)).astype(bf)
        maps.append({"xt_sb": xt_sb, "wqk_sb": wqk_sb, "wv_sb": wv_sb,
                     "pw_sb": pw_sb, "pb": pbarr})
    return maps


def kernel(x, qkv_w, proj_w, proj_b):
    runner = _get_runner()
    results = runner(make_in_maps(x, qkv_w, proj_w, proj_b))
    return np.stack([results[b]["out"] for b in range(N_CORES)],
                    axis=0).astype(np.float32)


# revision 57
# speedup vs baseline: 1.1842x; 1.1842x over previous
"""Multi-head attention block (QKV proj + softmax attention + out proj) on 8
Trainium2 NeuronCores, data-parallel over the batch dimension (one batch
element per core).

Self-contained: hardcodes shapes for x [8, 1024, 768], qkv_w [768, 2304],
proj_w [768, 768], proj_b [768]; returns [8, 1024, 768] float32.

Layout/schedule notes:
- All matmul operands are bf16 (host pre-converts); psum stays f32.
- Host pre-arranges every input into its SBUF image ([partition, ...]) so
  each load is one large fully-coalesced DMA.
- Per-head-pair software pipeline: window hp emits SC(hp+1) + AV(hp) + a
  slice of QK(hp+2) on the PE queue so the scalar-engine exp stream (the
  2nd-largest serial cost) hides completely behind matmuls.
- Softmax denominators ride as a 65th row of the V stationary tiles
  (ones-augmented), then reciprocal on DVE + gpsimd partition_broadcast;
  the normalize multiply reads the AV psum directly (no evacuation copy).
"""

import numpy as np
import ml_dtypes

import concourse.bass as bass
import concourse.mybir as mybir
import concourse.tile as tile
from concourse import bacc

N_CORES = 8
N = 1024          # tokens per batch element
C = 768           # model dim
H = 12            # heads
HD = 64           # head dim
CT = C // 128     # 6 contraction tiles
TT = N // 128     # 8 token tiles
HP = H // 2       # 6 head pairs
SCALE = HD ** -0.5

F32 = mybir.dt.float32
BF16 = mybir.dt.bfloat16


def _build():
    nc = bacc.Bacc("TRN2", target_bir_lowering=False, debug=False,
                   num_devices=N_CORES)
    xt = nc.dram_tensor("xt_sb", [128, CT, N], BF16, kind="ExternalInput").ap()
    wqk = nc.dram_tensor("wqk_sb", [128, CT, 2 * C], BF16, kind="ExternalInput").ap()
    wv = nc.dram_tensor("wv_sb", [128, CT, C], BF16, kind="ExternalInput").ap()
    pw = nc.dram_tensor("pw_sb", [128, CT, C], BF16, kind="ExternalInput").ap()
    pb = nc.dram_tensor("pb", [1, C], F32, kind="ExternalInput").ap()
    out = nc.dram_tensor("out", [N, C], BF16, kind="ExternalOutput").ap()

    with tile.TileContext(nc) as tc:
        _emit(nc, tc, xt, wqk, wv, pw, pb, out)
    nc.compile()
    return nc


def _emit(nc, tc, xt, wqk, wv, pw, pb, out):
    from contextlib import ExitStack
    ctx = ExitStack()
    with ctx:
        sb = ctx.enter_context(tc.tile_pool(name="sb", bufs=1))
        epool = ctx.enter_context(tc.tile_pool(name="ep", bufs=12))
        upool = ctx.enter_context(tc.tile_pool(name="up", bufs=2))
        npool = ctx.enter_context(tc.tile_pool(name="norm", bufs=2))
        opool = ctx.enter_context(tc.tile_pool(name="osb", bufs=2))
        ps = ctx.enter_context(tc.tile_pool(name="scps", bufs=2, space="PSUM"))
        av_ps = ctx.enter_context(tc.tile_pool(name="avps", bufs=1, space="PSUM"))
        dpool = ctx.enter_context(tc.tile_pool(name="drs", bufs=2, space="DRAM"))

        XT = sb.tile([128, CT, N], BF16, tag="XT")
        WQK = sb.tile([128, CT, 2 * C], BF16, tag="WQK")
        WV = sb.tile([128, CT, C], BF16, tag="WV")
        PWt = sb.tile([128, CT, C], BF16, tag="PW")
        QT = sb.tile([128, HP, N], BF16, tag="QT")
        KT = sb.tile([128, HP, N], BF16, tag="KT")
        VA = sb.tile([128, TT, H, HD + 1], BF16, tag="VA")
        OT = sb.tile([128, CT, N], BF16, tag="OT")
        PBB = sb.tile([128, C], F32, tag="PBB")

        # ---- input loads: two HW DMA queues (sync + scalar) in parallel.
        # The critical-path tensors (WQK slice 0, XT, WV) are split across
        # both queues up front; everything else trails on the scalar queue.
        nc.sync.dma_start(WQK[:, :, 0:256], wqk[:, :, 0:256])
        nc.sync.dma_start(XT[:], xt[:])
        nc.sync.dma_start(WV[:], wv[:])
        for hp in range(1, HP):
            nc.scalar.dma_start(WQK[:, :, hp * 256:(hp + 1) * 256],
                                wqk[:, :, hp * 256:(hp + 1) * 256])
        nc.scalar.dma_start(PWt[:], pw[:])
        pbsrc = pb[:, :]
        pb_b = bass.AP(tensor=pbsrc.tensor, offset=pbsrc.offset,
                       ap=[[0, 128]] + [list(a) for a in pbsrc.ap[1:]])
        nc.gpsimd.dma_start(PBB[:], pb_b)
        nc.vector.memset(
            VA[:, :, :, HD:HD + 1].rearrange("p t h o -> p (t h o)"), 1.0)

        e_tiles = {}

        def emit_qk_unit(hp, u):
            qk, qc = divmod(u, 2)
            dest = QT if qk == 0 else KT
            f0 = hp * 256 + qk * 128
            p = ps.tile([128, 512], F32, tag="sc", name="qkps")
            for ct in range(CT):
                nc.tensor.matmul(
                    p[:], lhsT=WQK[:, ct, f0:f0 + 128],
                    rhs=XT[:, ct, qc * 512:(qc + 1) * 512],
                    start=(ct == 0), stop=(ct == CT - 1))
            nc.vector.tensor_copy(dest[:, hp, qc * 512:(qc + 1) * 512], p[:])

        def emit_v_unit(tt, half):
            p = ps.tile([128, 384], F32, tag="sc", name="vps")
            for ct in range(CT):
                nc.tensor.matmul(
                    p[:], lhsT=XT[:, ct, tt * 128:(tt + 1) * 128],
                    rhs=WV[:, ct, half * 384:(half + 1) * 384],
                    start=(ct == 0), stop=(ct == CT - 1))
            nc.vector.tensor_copy(
                VA[:, tt, half * 6:(half + 1) * 6, 0:HD],
                p[:].rearrange("p (h d) -> p h d", d=HD))

        def emit_sc_pair(w, kt):
            # scores for both heads of pair w, packed [A-qc | B-qc] per tile:
            # A runs on PE rows 0-63, B on rows 64-127 — adjacent row-tiled
            # matmuls dual-stream (~1.7x measured vs serial)
            es = []
            for qc in range(2):
                pS = ps.tile([128, N], F32, tag="sc", name="scps")
                nc.tensor.matmul(
                    pS[:, 0:512],
                    lhsT=KT[0:64, w, kt * 128:(kt + 1) * 128],
                    rhs=QT[0:64, w, qc * 512:(qc + 1) * 512],
                    start=True, stop=True)
                nc.tensor.matmul(
                    pS[:, 512:1024],
                    lhsT=KT[64:128, w, kt * 128:(kt + 1) * 128],
                    rhs=QT[64:128, w, qc * 512:(qc + 1) * 512],
                    start=True, stop=True)
                e = epool.tile([128, N], BF16, tag="e", name="e")
                nc.scalar.activation(e[:], pS[:],
                                     mybir.ActivationFunctionType.Exp,
                                     scale=SCALE)
                es.append(e)
            e_tiles[(w, kt)] = es

        def emit_av_pair(w, kt, avA, avB):
            e0, e1 = e_tiles.pop((w, kt))
            for qc, e in ((0, e0), (1, e1)):
                nc.tensor.matmul(
                    avA[:, qc * 512:(qc + 1) * 512],
                    lhsT=VA[:, kt, 2 * w, :],
                    rhs=e[:, 0:512],
                    start=(kt == 0), stop=(kt == TT - 1))
                nc.tensor.matmul(
                    avB[:, qc * 512:(qc + 1) * 512],
                    lhsT=VA[:, kt, 2 * w + 1, :],
                    rhs=e[:, 512:1024],
                    start=(kt == 0), stop=(kt == TT - 1))

        def emit_norm(h, av):
            hp, half = divmod(h, 2)
            if h == H - 1:
                # last head is latency-critical (proj finals wait on it):
                # skip the DMA transpose round-trip; reciprocal straight off
                # the psum row, broadcast across partitions on gpsimd
                d0 = npool.tile([1, N], F32, tag="d0", name="d0")
                nc.vector.tensor_copy(d0[:], av[HD:HD + 1, :])
                U = upool.tile([HD + 1, N], F32, tag="U", name="U")
                nc.vector.tensor_copy(U[:], av[:])
                Db = npool.tile([64, N], F32, tag="Db", name="Db")
                nc.gpsimd.partition_broadcast(Db[:], d0[0:1, :])
                bc = npool.tile([64, N], F32, tag="bc", name="bc")
                # fast variant (~2^-12 rel err, plenty here): one DVE pass
                # less on the latency-critical last-head chain
                nc.vector.reciprocal_approx_fast(out=bc[:], in_=Db[:])
                for qc in range(2):
                    nc.vector.tensor_mul(
                        OT[half * 64:(half + 1) * 64, hp,
                           qc * 512:(qc + 1) * 512],
                        U[0:HD, qc * 512:(qc + 1) * 512],
                        bc[:, qc * 512:(qc + 1) * 512])
                return
            # evacuate psum right away so the next window's AV can allocate;
            # the reciprocal chain then runs from SBUF asynchronously
            U = upool.tile([HD + 1, N], F32, tag="U", name="U")
            nc.vector.tensor_copy(U[:], av[:])
            dscr = dpool.tile([N], F32, tag="dscr", name="dscr")
            nc.sync.dma_start(dscr[:], U[HD:HD + 1, :])
            Dt = npool.tile([64, 16], F32, tag="Dt", name="Dt")
            nc.sync.dma_start(Dt[:], dscr[:].rearrange("(p j) -> p j", j=16))
            Rt = npool.tile([64, 16], F32, tag="Rt", name="Rt")
            scr = npool.tile([64, 16], F32, tag="scr", name="scr")
            nc.vector.reciprocal_approx_accurate(Rt[:], Dt[:], scr[:])
            rscr = dpool.tile([N], F32, tag="rscr", name="rscr")
            nc.sync.dma_start(rscr[:].rearrange("(p j) -> p j", j=16), Rt[:])
            bc = npool.tile([64, N], F32, tag="bc", name="bc")
            rs = rscr[:]
            bcast = bass.AP(tensor=rs.tensor, offset=rs.offset,
                            ap=[[0, 64]] + [list(a) for a in rs.ap])
            nc.sync.dma_start(bc[:], bcast)
            for qc in range(2):
                nc.vector.tensor_mul(
                    OT[half * 64:(half + 1) * 64, hp, qc * 512:(qc + 1) * 512],
                    U[0:HD, qc * 512:(qc + 1) * 512],
                    bc[:, qc * 512:(qc + 1) * 512])

        osb_tiles = {}

        def _pj_views(p, osb):
            # matmul outs go to cols [0:384] and [512:896] (bank-aligned);
            # one strided DVE add folds both into the packed osb layout
            pv = p[:, 0:1024].rearrange("p (b k) -> p b k", b=2)[:, :, 0:384]
            ov = osb[:, 0:768].rearrange("p (b k) -> p b k", b=2)
            return pv, ov

        def emit_proj_partial(tt):
            # accumulate proj over ct 0..4 (heads 0..9, normalized well before
            # the last window) into SBUF, leaving only ct 5 for the tail
            osb = osb_tiles[tt] = opool.tile([128, C], BF16, tag="osb",
                                             name="osb", bufs=TT)
            p = ps.tile([128, N], F32, tag="sc", name="pjps")
            for nch in range(2):
                for ct in range(CT - 1):
                    nc.tensor.matmul(
                        p[:, nch * 512:nch * 512 + 384],
                        lhsT=OT[:, ct, tt * 128:(tt + 1) * 128],
                        rhs=PWt[:, ct, nch * 384:(nch + 1) * 384],
                        start=(ct == 0), stop=(ct == CT - 2))
            pv, ov = _pj_views(p, osb)
            bv = PBB[:, 0:768].rearrange("p (b k) -> p b k", b=2)
            nc.vector.tensor_add(ov, pv, bv)

        def emit_proj_final(tt, q1):
            osb = osb_tiles[tt]
            # alternate between the two free psum rings so final matmuls
            # pipeline deeper instead of stalling behind each DVE add
            pool, tag = (ps, "sc") if tt % 2 == 0 else (av_ps, "avA")
            p = pool.tile([128, N], F32, tag=tag, name="pjps")
            for nch in range(2):
                nc.tensor.matmul(
                    p[:, nch * 512:nch * 512 + 384],
                    lhsT=OT[:, CT - 1, tt * 128:(tt + 1) * 128],
                    rhs=PWt[:, CT - 1, nch * 384:(nch + 1) * 384],
                    start=True, stop=True)
            pv, ov = _pj_views(p, osb)
            nc.vector.tensor_add(ov, ov, pv)
            eng = nc.sync if q1 else nc.scalar
            eng.dma_start(out[tt * 128:(tt + 1) * 128, :], osb[:])

        # filler units per (window, kt-group): V projections + next QK
        # projections slotted into PE slack so no phase runs standalone.
        # Deadlines: V half-0 unit tt before AV(0,tt); V half-1 before w3;
        # QK(w+1) during window w; proj partials after norm(9) = window 4.
        # group TT (=8) is a boundary slot consumed after the window's AV
        # drain: it papers over the exp(kt7)->SC(next,kt0) ring bubble
        fillers = {}
        for kt in range(TT):
            fillers[(0, kt)] = [lambda kt=kt: emit_v_unit(kt, 0)]
            if kt % 2 == 1 and kt < 7:
                fillers[(0, kt)].append(
                    lambda u=kt // 2: emit_qk_unit(1, u))
        fillers[(0, TT)] = [lambda: emit_qk_unit(1, 3)]
        for w in (1, 2):
            for j in range(4):
                fillers.setdefault((w, 2 * j + 1), []).append(
                    lambda tt=4 * (w - 1) + j: emit_v_unit(tt, 1))
            for j in range(3):
                fillers.setdefault((w, 2 * j), []).append(
                    lambda hp=w + 1, u=j: emit_qk_unit(hp, u))
            fillers[(w, TT)] = [lambda hp=w + 1: emit_qk_unit(hp, 3)]
        for w in (3, 4):
            for j in range(3):
                fillers.setdefault((w, 2 * j), []).append(
                    lambda hp=w + 1, u=j: emit_qk_unit(hp, u))
            fillers[(w, TT)] = [lambda hp=w + 1: emit_qk_unit(hp, 3)]
        for j in range(4):
            fillers.setdefault((5, 4 + j), []).append(
                lambda tt=j: emit_proj_partial(tt))

        # ---- schedule: PE warm-up, lead-in QK(0), one head-pair/window ----
        # dummy matmuls while the input DMAs land: the PE clock needs ~3us
        # of continuous work to ramp to full speed, so burn the DMA wait on
        # throwaway accumulates instead of ramping inside the first window
        warm = sb.tile([128, 512], BF16, tag="warm")
        nc.vector.memset(warm[:], 0.25)
        wp = ps.tile([128, 512], F32, tag="sc", name="warmps")
        for i in range(26):
            # 64-row contraction: keeps the clock ramping at half the MAC
            # power (sustained full-array warm work trips the util throttle)
            nc.tensor.matmul(wp[:], lhsT=warm[0:64, 0:128], rhs=warm[0:64, :],
                             start=(i == 0), stop=(i == 25))
        for u in range(4):
            emit_qk_unit(0, u)
        for w in range(HP):
            avA = av_ps.tile([HD + 1, N], F32, tag="avA", name="avA")
            avB = av_ps.tile([HD + 1, N], F32, tag="avB", name="avB")
            pend = []
            for kt in range(TT):
                emit_sc_pair(w, kt)
                pend.append(kt)
                if len(pend) > 2:
                    emit_av_pair(w, pend.pop(0), avA, avB)
                for f in fillers.get((w, kt), ()):
                    f()
            for kt in pend:
                emit_av_pair(w, kt, avA, avB)
            for f in fillers.get((w, TT), ()):
                f()
            emit_norm(2 * w, avA)
            emit_norm(2 * w + 1, avB)
        for tt in range(4, TT):
            emit_proj_partial(tt)
        for tt in range(TT):
            emit_proj_final(tt, tt % 2 == 0)


_CACHE = {}


def _get_runner():
    """Build + compile once; return a callable(in_maps) -> list of out dicts.

    Keeps a persistent jitted shard_map executable so repeat calls skip
    retracing/recompiling (mirrors bass2jax.run_bass_via_pjrt).
    """
    if "runner" in _CACHE:
        return _CACHE["runner"]

    import jax
    from jax.experimental.shard_map import shard_map
    from jax.sharding import Mesh, PartitionSpec
    from concourse import bass2jax

    nc = _build()
    bass2jax.install_neuronx_cc_hook()

    partition_name = (nc.partition_id_tensor.name if nc.partition_id_tensor
                      else None)
    in_names, out_names, out_avals, zero_outs = [], [], [], []
    for alloc in nc.m.functions[0].allocations:
        if not isinstance(alloc, mybir.MemoryLocationSet):
            continue
        name = alloc.memorylocations[0].name
        if alloc.kind == "ExternalInput":
            if name != partition_name:
                in_names.append(name)
        elif alloc.kind == "ExternalOutput":
            out_names.append(name)
            shape = tuple(alloc.tensor_shape)
            dtype = mybir.dt.np(alloc.dtype)
            out_avals.append(jax.core.ShapedArray(shape, dtype))
            zero_outs.append(np.zeros(shape, dtype))
    n_params = len(in_names)
    n_outs = len(out_avals)
    all_in_names = list(in_names) + list(out_names)
    if partition_name is not None:
        all_in_names.append(partition_name)
    donate = tuple(range(n_params, n_params + n_outs))

    def _body(*args):
        operands = list(args)
        if partition_name is not None:
            operands.append(bass2jax.partition_id_tensor())
        outs = bass2jax._bass_exec_p.bind(
            *operands,
            out_avals=tuple(out_avals),
            in_names=tuple(all_in_names),
            out_names=tuple(out_names),
            lowering_input_output_aliases=(),
            sim_require_finite=True,
            sim_require_nnan=True,
            nc=nc,
        )
        return tuple(outs)

    devices = jax.devices()[:N_CORES]
    mesh = Mesh(np.asarray(devices), ("core",))
    in_specs = (PartitionSpec("core"),) * (n_params + n_outs)
    out_specs = (PartitionSpec("core"),) * n_outs
    sharded = jax.jit(
        shard_map(_body, mesh=mesh, in_specs=in_specs, out_specs=out_specs,
                  check_rep=False),
        donate_argnums=donate, keep_unused=True)

    def runner(in_maps):
        concat_in = [
            np.concatenate([np.asarray(m[name]) for m in in_maps], axis=0)
            for name in in_names
        ]
        concat_zeros = [
            np.zeros((N_CORES * z.shape[0], *z.shape[1:]), z.dtype)
            for z in zero_outs
        ]
        out_arrs = sharded(*concat_in, *concat_zeros)
        return [
            {name: np.asarray(out_arrs[i]).reshape(N_CORES, *out_avals[i].shape)[c]
             for i, name in enumerate(out_names)}
            for c in range(N_CORES)
        ]

    _CACHE["runner"] = runner
    _CACHE["nc"] = nc
    return runner


def make_in_maps(x, qkv_w, proj_w, proj_b):
    bf = ml_dtypes.bfloat16
    qkv = np.asarray(qkv_w, np.float32)
    q = qkv[:, 0:C].reshape(CT, 128, HP, 128)
    k = qkv[:, C:2 * C].reshape(CT, 128, HP, 128)
    wqk_sb = np.empty((128, CT, 2 * C), np.float32)
    for hp in range(HP):
        wqk_sb[:, :, hp * 256:hp * 256 + 128] = q[:, :, hp, :].transpose(1, 0, 2)
        wqk_sb[:, :, hp * 256 + 128:hp * 256 + 256] = \
            k[:, :, hp, :].transpose(1, 0, 2)
    wqk_sb = np.ascontiguousarray(wqk_sb).astype(bf)
    wv_sb = np.ascontiguousarray(
        qkv[:, 2 * C:3 * C].reshape(CT, 128, C).transpose(1, 0, 2)).astype(bf)
    pw_sb = np.ascontiguousarray(
        np.asarray(proj_w, np.float32).reshape(CT, 128, C)
        .transpose(1, 0, 2)).astype(bf)
    pbarr = np.asarray(proj_b, np.float32).reshape(1, C)
    maps = []
    for b in range(N_CORES):
        xt_sb = np.ascontiguousarray(
            np.asarray(x[b], np.float32).T.reshape(CT, 128, N)
            .transpose(1, 0, 2)).astype(bf)
        maps.append({"xt_sb": xt_sb, "wqk_sb": wqk_sb, "wv_sb": wv_sb,
                     "pw_sb": pw_sb, "pb": pbarr})
    return maps


def kernel(x, qkv_w, proj_w, proj_b):
    runner = _get_runner()
    results = runner(make_in_maps(x, qkv_w, proj_w, proj_b))
    return np.stack([results[b]["out"] for b in range(N_CORES)],
                    axis=0).astype(np.float32)
